# revision 7
# baseline (speedup 1.0000x reference)
"""Causal multi-head attention on 8 Trainium2 NeuronCores.

Problem: B=2, NH=16, T=2048, D=64 fp32.
Sharding: the 32 (batch, head) pairs are split 4-per-core; each core runs its
heads' full causal attention independently (no collectives).

Per-core kernel design (per head):
  - S^T blocks [k=128 partitions, q free] = K_blk @ Q^T via PE (float32r).
    Causality at 128-row granularity: iteration kb only computes q >= 128*kb.
  - Diagonal 128x128 block gets an additive -1e9 upper-strict-triangle mask
    (DVE, in-place in PSUM).
  - exp(S/8) on ScalarE, PSUM -> SBUF (this doubles as the PSUM evacuation).
  - O^T accumulation: PSUM [65, q] += [V | ones]^T_blk @ P^T_blk. Row 64 is
    the softmax denominator (free).
  - Epilogue: copy O^T to SBUF, PE-transpose 128-col chunks back to [q, 65],
    multiply by reciprocal of the sums column (DVE), DMA out.

The host side only reformats layouts (transpose/pack/shard in numpy); every
FLOP of the attention math runs on device.
"""

import numpy as np

import concourse.mybir as mybir
import concourse.tile as tile
from concourse import bacc
from concourse.bass_utils import run_bass_kernel_spmd

B, NH, T, D = 2, 16, 2048, 64
HPC = 4  # heads per core
NCORES = 8
NKB = T // 128  # 16 k-blocks of 128 rows
F32 = mybir.dt.float32
F32R = mybir.dt.float32r
NEG = -1.0e9

_cached = {}


def _build():
    if "nc" in _cached:
        return _cached["nc"]
    nc = bacc.Bacc("TRN2", target_bir_lowering=False, debug=False)
    # Q^T / K^T: [64, T] (d on partitions)
    qt = nc.dram_tensor("qt", (HPC, D, T), F32R, kind="ExternalInput").ap()
    kt = nc.dram_tensor("kt", (HPC, D, T), F32R, kind="ExternalInput").ap()
    # V augmented with a ones column: [h, p, c, d] = V[h, 128*c + p, d], d=64 -> 1.0
    v = nc.dram_tensor("v", (HPC, 128, NKB, D + 1), F32R, kind="ExternalInput").ap()
    mask = nc.dram_tensor("mask", (128, 128), F32, kind="ExternalInput").ap()
    ident = nc.dram_tensor("ident", (128, 128), F32, kind="ExternalInput").ap()
    # out [h, p, c*64 + d] = O[h, 128*c + p, d]
    o = nc.dram_tensor("o", (HPC, 128, NKB * D), F32, kind="ExternalOutput").ap()

    EXP = mybir.ActivationFunctionType.Exp

    with tile.TileContext(nc) as tc:
        with (
            tc.tile_pool(name="constp", bufs=1) as constp,
            tc.tile_pool(name="qkp", bufs=2) as qkp,
            tc.tile_pool(name="ptp", bufs=3) as ptp,
            tc.tile_pool(name="osbp", bufs=2) as osbp,
            tc.tile_pool(name="spp", bufs=2, space="PSUM") as spp,
            tc.tile_pool(name="opp", bufs=1, space="PSUM") as opp,
        ):
            mask_sb = constp.tile([128, 128], F32)
            nc.sync.dma_start(mask_sb[:], mask[:])
            id_sb = constp.tile([128, 128], F32)
            nc.sync.dma_start(id_sb[:], ident[:])

            for h in range(HPC):
                qt_sb = qkp.tile([D, T], F32R, tag="qt")
                nc.sync.dma_start(qt_sb[:], qt[h])
                kt_sb = qkp.tile([D, T], F32R, tag="kt")
                nc.sync.dma_start(kt_sb[:], kt[h])
                v_sb = qkp.tile([128, NKB, D + 1], F32R, tag="v")
                nc.sync.dma_start(v_sb[:], v[h])

                oacc = opp.tile([D + 1, T], F32, tag="oacc")

                for kb in range(NKB):
                    qs = kb * 128
                    kslice = kt_sb[:, qs : qs + 128]
                    for g in range(qs // 1024, 2):
                        c0 = max(qs, 1024 * g)
                        c1 = 1024 * (g + 1)
                        lo = c0 - 1024 * g
                        sch = spp.tile([128, 1024], F32, tag="s")
                        p = c0
                        while p < c1:
                            pe = min(c1, (p // 512 + 1) * 512)
                            qslice = qt_sb[:, p:pe]
                            nc.tensor.matmul(
                                sch[:, p - 1024 * g : pe - 1024 * g],
                                lhsT=kslice,
                                rhs=qslice,
                                start=True,
                                stop=True,
                            )
                            p = pe
                        if c0 == qs:
                            nc.vector.tensor_add(
                                sch[:, lo : lo + 128], sch[:, lo : lo + 128], mask_sb[:]
                            )
                        ptt = ptp.tile([128, 1024], F32R, tag="pt")
                        nc.scalar.activation(
                            ptt[:, lo:1024], sch[:, lo:1024], EXP, scale=0.125
                        )
                        p = c0
                        while p < c1:
                            pe = min(c1, (p // 512 + 1) * 512)
                            nc.tensor.matmul(
                                oacc[:, p:pe],
                                lhsT=v_sb[:, kb, :],
                                rhs=ptt[:, p - 1024 * g : pe - 1024 * g],
                                start=(kb == 0),
                                stop=(kb == (pe - 1) // 128),
                                skip_group_check=True,
                            )
                            p = pe

                # Epilogue: O = (O^T / sums)^T
                ot_sb = osbp.tile([D + 1, T], F32, tag="ot")
                nc.vector.tensor_copy(ot_sb[:], oacc[:])
                o_sb = osbp.tile([128, NKB * D], F32, tag="o")
                rec = osbp.tile([128, NKB], F32, tag="rec")
                for g4 in range(4):
                    pso = spp.tile([128, 1024], F32, tag="s")
                    for j in range(4):
                        c = 4 * g4 + j
                        nc.tensor.transpose(
                            pso[:, 65 * j : 65 * j + 65],
                            ot_sb[:, 128 * c : 128 * c + 128],
                            id_sb[:65, :65],
                        )
                    sums = pso[:, : 4 * 65].rearrange("p (c d) -> p c d", d=65)[:, :, 64]
                    nc.vector.reciprocal(rec[:, 4 * g4 : 4 * g4 + 4], sums)
                    for j in range(4):
                        c = 4 * g4 + j
                        nc.vector.tensor_scalar_mul(
                            o_sb[:, 64 * c : 64 * c + 64],
                            pso[:, 65 * j : 65 * j + 64],
                            rec[:, c : c + 1],
                        )
                nc.sync.dma_start(o[h], o_sb[:])

    nc.compile()
    _cached["nc"] = nc
    return nc


def _prep_in_maps(Q, K, V):
    Q = np.asarray(Q, dtype=np.float32).reshape(B * NH, T, D)
    K = np.asarray(K, dtype=np.float32).reshape(B * NH, T, D)
    V = np.asarray(V, dtype=np.float32).reshape(B * NH, T, D)

    mask = np.where(
        np.arange(128)[:, None] <= np.arange(128)[None, :], 0.0, NEG
    ).astype(np.float32)
    ident = np.eye(128, dtype=np.float32)

    in_maps = []
    for c in range(NCORES):
        hs = slice(HPC * c, HPC * (c + 1))
        qt = Q[hs].transpose(0, 2, 1)  # [hpc, 64, T]
        kt = K[hs].transpose(0, 2, 1)
        va = np.concatenate(
            [V[hs], np.ones((HPC, T, 1), dtype=np.float32)], axis=-1
        )  # [hpc, T, 65]
        va = va.reshape(HPC, NKB, 128, D + 1).transpose(0, 2, 1, 3)  # [hpc,128,16,65]
        in_maps.append(
            {
                "qt": np.ascontiguousarray(qt),
                "kt": np.ascontiguousarray(kt),
                "v": np.ascontiguousarray(va),
                "mask": mask,
                "ident": ident,
            }
        )
    return in_maps


def _gather(results):
    out = np.empty((B * NH, T, D), dtype=np.float32)
    for c in range(NCORES):
        oc = results[c]["o"]  # [HPC, 128, NKB*D]
        for s in range(HPC):
            out[HPC * c + s] = (
                oc[s].reshape(128, NKB, D).transpose(1, 0, 2).reshape(T, D)
            )
    return out.reshape(B, NH, T, D)


def _run(in_maps, **kwargs):
    nc = _build()
    return run_bass_kernel_spmd(nc, in_maps, core_ids=list(range(NCORES)), **kwargs)


def kernel(Q, K, V):
    in_maps = _prep_in_maps(Q, K, V)
    res = _run(in_maps)
    return _gather(res.results)
